# revision 5
# baseline (speedup 1.0000x reference)
"""Trainium2 Bass kernel — weight-composition + analytic-BN, latency-optimized.

Math (per core = one batch):
    A = phi_x g_x^T accumulated on PE from per-chunk projections (never
    materializes the L x L score matrix); y = (A theta_x)/L collapses to
    M_y x + ty; W projection composes to M_w = W_w M_y (256x256).
    BN statistics are computed analytically pre-collective:
      S1 = Ww(u + L ty), Q = diag(Ww Y2q Ww^T) + s2*(s1+S1),
    with Y2q = As^T Theta As from the theta gram. One AllGather of
    [4,128]-transposed per-core stats; everything after is local.

Schedule findings baked in (from perfetto traces):
  - dma_start issues are credit-limited and block the issuing engine ->
    all 16 x pieces on sync (no pre-CC compute), weights on scalar's ring,
    all tail out-DMAs on sync
  - make_identity first on gpsimd (it gates the PE weight transposes);
    weights cast to bf16 on vector so the transposes run at 2x PE rate
  - casts both on scalar (ring-paced); projection-PSUM evicts on vector
  - AllGather payload transposed to [4,128] so the gathered readback is 32
    contiguous 512B rows (not 1024 16B gathers); reduced with a selector
    matmul + transpose on PE
  - phase 2 (Wy = M_w x, bf16) overlaps the collective
  - tail: out = diag(a)*Wy + I*x on the PE (PSUM accumulate), bias folded
    into the eviction; out DMA as 2 partition-split [64,512] pieces/unit
"""

import sys

if "/opt/trn_rl_repo" not in sys.path:
    sys.path.insert(0, "/opt/trn_rl_repo")

import numpy as np

import concourse.bass as bass
import concourse.mybir as mybir
import concourse.tile as tile
from concourse import bacc
from concourse.bass_utils import run_bass_kernel_spmd
from concourse.masks import make_identity

B, C, L, OC = 8, 256, 4096, 128
CH = 512
NCH = L // CH
SUB = 128
NSUB = CH // SUB
EPS = 1e-5
N_CORES = 8

f32 = mybir.dt.float32
bf16 = mybir.dt.bfloat16
AX = mybir.AxisListType
AF = mybir.ActivationFunctionType
ALU = mybir.AluOpType

# x DMA pieces per c-chunk: (start_col, n_cols), one per 512-col chunk
X_PIECES = [(k * 512, 512) for k in range(8)]


def build_nc():
    nc = bacc.Bacc(
        "TRN2", target_bir_lowering=False, debug=False, num_devices=N_CORES
    )

    x_e = nc.declare_dram_parameter("x", [C, L], f32, isOutput=False)
    thw_e = nc.declare_dram_parameter("theta_w", [OC, C], f32, isOutput=False)
    thb_e = nc.declare_dram_parameter("theta_b", [OC, 1], f32, isOutput=False)
    phw_e = nc.declare_dram_parameter("phi_w", [OC, C], f32, isOutput=False)
    phb_e = nc.declare_dram_parameter("phi_b", [1, OC], f32, isOutput=False)
    gw_e = nc.declare_dram_parameter("g_w", [OC, C], f32, isOutput=False)
    gb_e = nc.declare_dram_parameter("g_b", [1, OC], f32, isOutput=False)
    Ww_e = nc.declare_dram_parameter("W_w", [C, OC], f32, isOutput=False)
    gam_e = nc.declare_dram_parameter("gamma", [C, 1], f32, isOutput=False)
    bet_e = nc.declare_dram_parameter("beta", [C, 1], f32, isOutput=False)
    out_e = nc.declare_dram_parameter("out", [C, L], f32, isOutput=True)

    with tile.TileContext(nc) as tc:
        with (
            tc.tile_pool(name="const", bufs=1) as cp,
            tc.tile_pool(name="big", bufs=1) as bp,
            tc.tile_pool(name="work", bufs=3) as wp,
            tc.tile_pool(name="dram", bufs=1, space="DRAM") as dp,
            tc.tile_pool(name="ps_setup", bufs=1, space="PSUM") as sp,
        ):
            # ---- x pieces first: c0 on sync, c1 on scalar ------------------
            xf = [bp.tile([128, L], f32, tag=f"xf{c}", name=f"xf{c}") for c in range(2)]
            xb = [bp.tile([128, L], bf16, tag=f"xb{c}", name=f"xb{c}") for c in range(2)]
            thw = cp.tile([OC, C], f32, tag="thw")
            phw = cp.tile([OC, C], f32, tag="phw")
            gw = cp.tile([OC, C], f32, tag="gw")
            Ww = [cp.tile([128, OC], f32, tag=f"Ww{c}", name=f"Ww{c}") for c in range(2)]

            # dma_start issues are credit-limited and block the issuing
            # engine until the ring drains -> keep ALL x issues on sync
            # (which has no pre-CC compute), weights on scalar's ring (small,
            # drains fast), and the last chunk pair on gpsimd's SWDGE.
            thbf = cp.tile([OC, 2], f32, tag="thbf")
            nc.sync.dma_start(thbf[:, 0:1], thb_e[:, :])
            nc.scalar.dma_start(thw[:], thw_e[:, :])
            nc.scalar.dma_start(phw[:], phw_e[:, :])
            nc.scalar.dma_start(gw[:], gw_e[:, :])
            nc.scalar.dma_start(Ww[0][:], Ww_e[0:128, :])
            nc.scalar.dma_start(Ww[1][:], Ww_e[128:256, :])
            for s0, ncol in X_PIECES:
                for c in range(2):
                    psl = slice(c * 128, (c + 1) * 128)
                    nc.sync.dma_start(
                        xf[c][:, s0 : s0 + ncol], x_e[psl, s0 : s0 + ncol]
                    )
            # ---- gpsimd: identity + tiny params ----------------------------
            ident = cp.tile([128, 128], f32, tag="ident")
            make_identity(nc, ident[:])
            ident_b = cp.tile([128, 128], bf16, tag="ident_b")
            nc.gpsimd.tensor_copy(ident_b[:], ident[:])
            epsc = cp.tile([128, 1], f32, tag="epsc")
            nc.gpsimd.memset(epsc[:], EPS)
            id4_d = dp.tile([4, 4], f32, name="id4_d")
            nc.gpsimd.dma_start(id4_d[:], ident[0:4, 0:4])
            sel32 = cp.tile([32, 4], f32, tag="sel32")
            for r in range(8):
                nc.gpsimd.dma_start(sel32[4 * r : 4 * r + 4, :], id4_d[:])

            pgbf = cp.tile([1, 3 * OC], f32, tag="pgbf")
            nc.vector.memset(pgbf[:], 0.0)
            nc.gpsimd.dma_start(pgbf[0:1, OC : 2 * OC], phb_e[:, :])
            nc.gpsimd.dma_start(pgbf[0:1, 2 * OC : 3 * OC], gb_e[:, :])
            gam2 = cp.tile([128, 2], f32, tag="gam2")
            nc.gpsimd.dma_start(
                gam2[:, :], gam_e[:, :].rearrange("(c p) one -> p (c one)", p=128)
            )
            bet2 = cp.tile([128, 2], f32, tag="bet2")
            nc.gpsimd.dma_start(
                bet2[:, :], bet_e[:, :].rearrange("(c p) one -> p (c one)", p=128)
            )

            # ---- derived small tiles (gpsimd, off critical path) -----------
            nc.gpsimd.tensor_copy(thbf[:, 1:2], thbf[:, 0:1])
            thb2 = cp.tile([OC, 2], bf16, tag="thb2")
            nc.gpsimd.tensor_copy(thb2[:], thbf[:])
            pg_bias = cp.tile([1, 3 * OC], bf16, tag="pg_bias")
            nc.gpsimd.tensor_copy(pg_bias[:], pgbf[:])
            gbL = cp.tile([1, OC], bf16, tag="gbL")
            nc.gpsimd.tensor_scalar_mul(gbL[0:1, :], pgbf[0:1, 2 * OC : 3 * OC], float(L))
            thw_b = cp.tile([OC, C], bf16, tag="thw_b")
            nc.gpsimd.tensor_copy(thw_b[:], thw[:])

            # ---- transposes in bf16 (2x PE rate): cast weights on vector ---
            phw_b = cp.tile([OC, C], bf16, tag="phw_b")
            nc.vector.tensor_copy(phw_b[:], phw[:])
            gw_b = cp.tile([OC, C], bf16, tag="gw_b")
            nc.vector.tensor_copy(gw_b[:], gw[:])
            Ww_b = [
                cp.tile([128, OC], bf16, tag=f"Ww_b{c}", name=f"Ww_b{c}")
                for c in range(2)
            ]
            for c in range(2):
                nc.vector.tensor_copy(Ww_b[c][:], Ww[c][:])
            rhsT = [
                cp.tile([128, 3 * OC], bf16, tag=f"rhsT{c}", name=f"rhsT{c}")
                for c in range(2)
            ]
            WwT = cp.tile([128, C], bf16, tag="WwT")
            for c in range(2):
                csl = slice(c * 128, (c + 1) * 128)
                t1 = sp.tile([128, 128], bf16, tag="tb")
                nc.tensor.transpose(t1[:], thw_b[:, csl], ident_b[:])
                nc.vector.tensor_copy(rhsT[c][:, 0:OC], t1[:])
                t2 = sp.tile([128, 128], bf16, tag="tb")
                nc.tensor.transpose(t2[:], phw_b[:, csl], ident_b[:])
                nc.vector.tensor_copy(rhsT[c][:, OC : 2 * OC], t2[:])
                t3 = sp.tile([128, 128], bf16, tag="tb")
                nc.tensor.transpose(t3[:], gw_b[:, csl], ident_b[:])
                nc.vector.tensor_copy(rhsT[c][:, 2 * OC : 3 * OC], t3[:])
            for c in range(2):
                t4 = sp.tile([128, 128], bf16, tag="tb")
                nc.tensor.transpose(t4[:], Ww_b[c][:], ident_b[:])
                nc.vector.tensor_copy(WwT[:, c * 128 : (c + 1) * 128], t4[:])

            # ---- casts + row sums: c0 on scalar, c1 on vector --------------
            sxacc = [
                cp.tile([128, NCH], f32, tag=f"sxacc{c}", name=f"sxacc{c}")
                for c in range(2)
            ]
            for k in range(NCH):
                sl = slice(k * CH, (k + 1) * CH)
                for c in range(2):
                    nc.scalar.activation(
                        xb[c][:, sl], xf[c][:, sl], AF.Identity,
                        accum_out=sxacc[c][:, k : k + 1],
                    )
            sxf = [cp.tile([128, 2], f32, tag=f"sxf{c}", name=f"sxf{c}") for c in range(2)]
            sxb = [cp.tile([128, 2], bf16, tag=f"sxb{c}", name=f"sxb{c}") for c in range(2)]
            for c in range(2):
                nc.vector.reduce_sum(sxf[c][:, 0:1], sxacc[c][:, :], axis=AX.X)
                nc.vector.tensor_copy(sxf[c][:, 1:2], sxf[c][:, 0:1])
                nc.vector.tensor_copy(sxb[c][:], sxf[c][:])

            sth = cp.tile([128, 2], bf16, tag="sth")
            As = cp.tile([128, 128], bf16, tag="As")
            Th_sb = cp.tile([128, 128], bf16, tag="Th_sb")

            Wy = [bp.tile([128, L], bf16, tag=f"Wy{c}", name=f"Wy{c}") for c in range(2)]

            # ---- phase 1: projections + A/Theta grams ----------------------
            with (
                tc.tile_pool(name="pt", bufs=3, space="PSUM") as pt,
                tc.tile_pool(name="pa", bufs=1, space="PSUM") as pa,
                tc.tile_pool(name="pth", bufs=1, space="PSUM") as pth,
            ):
                A_ps = pa.tile([128, 128], f32, tag="A")
                Th_ps = pth.tile([128, 128], f32, tag="Th")
                NSUBT = NCH * NSUB
                LAG = 2
                pgs = {}
                for i in range(NSUBT + LAG):
                    if i < NSUBT:
                        lsl = slice(i * SUB, (i + 1) * SUB)
                        q = pt.tile([128, 3 * OC], f32, tag="pt")
                        nc.tensor.matmul(
                            q[:], xb[0][:, lsl], rhsT[0][:, :], start=True, stop=False
                        )
                        nc.tensor.matmul(
                            q[:], xb[1][:, lsl], rhsT[1][:, :], start=False, stop=True
                        )
                        pg = wp.tile([128, 3 * OC], bf16, tag="phigT", bufs=4)
                        nc.vector.tensor_copy(pg[:], q[:])
                        pgs[i] = pg
                    j = i - LAG
                    if j >= 0:
                        pg = pgs.pop(j)
                        first = j == 0
                        last = j == NSUBT - 1
                        nc.tensor.matmul(
                            A_ps[:], pg[:, OC : 2 * OC], pg[:, 2 * OC : 3 * OC],
                            start=first, stop=False,
                        )
                        nc.tensor.matmul(
                            Th_ps[:], pg[:, 0:OC], pg[:, 0:OC],
                            start=first, stop=last,
                        )

                # rank-1 bias corrections for A
                urow_sb = cp.tile([1, 2 * OC], bf16, tag="urow")
                urow_ps = sp.tile([1, 2 * OC], f32, tag="t")
                nc.tensor.matmul(
                    urow_ps[:], sxb[0][:, 0:1], rhsT[0][:, OC : 3 * OC],
                    start=True, stop=False,
                )
                nc.tensor.matmul(
                    urow_ps[:], sxb[1][:, 0:1], rhsT[1][:, OC : 3 * OC],
                    start=False, stop=True,
                )
                nc.scalar.copy(urow_sb[:], urow_ps[:])
                sth_ps = sp.tile([128, 2], f32, tag="t")
                nc.tensor.matmul(
                    sth_ps[:], rhsT[0][:, 0:OC], sxb[0][:], start=True, stop=False
                )
                nc.tensor.matmul(
                    sth_ps[:], rhsT[1][:, 0:OC], sxb[1][:], start=False, stop=True
                )
                nc.scalar.copy(sth[:], sth_ps[:])
                nc.tensor.matmul(
                    A_ps[:], pg_bias[0:1, OC : 2 * OC], urow_sb[0:1, OC : 2 * OC],
                    start=False, stop=False,
                )
                nc.tensor.matmul(
                    A_ps[:], urow_sb[0:1, 0:OC], pg_bias[0:1, 2 * OC : 3 * OC],
                    start=False, stop=False,
                )
                nc.tensor.matmul(
                    A_ps[:], pg_bias[0:1, OC : 2 * OC], gbL[0:1, :],
                    start=False, stop=True,
                )
                nc.scalar.mul(As[:], A_ps[:], 1.0 / L)
                nc.vector.tensor_copy(Th_sb[:], Th_ps[:])

            # ---- fused stats chain -----------------------------------------
            ZU = cp.tile([128, 132], bf16, tag="ZU")
            nc.scalar.copy(ZU[:, 128:130], sth[:])
            nc.gpsimd.tensor_copy(ZU[:, 130:132], thb2[:])
            YU = cp.tile([128, 132], bf16, tag="YU")
            sc8 = cp.tile([128, 8], f32, tag="sc8")
            qdiag = cp.tile([128, 2], f32, tag="qdiag")
            stats = cp.tile([128, 4], f32, tag="stats")

            M_y = cp.tile([128, C], bf16, tag="M_y")
            M_wT = [
                cp.tile([128, C], bf16, tag=f"M_wT{c}", name=f"M_wT{c}")
                for c in range(2)
            ]

            with tc.tile_pool(name="pc", bufs=2, space="PSUM") as pc:
                z_ps = pc.tile([128, 128], f32, tag="pc128")
                nc.tensor.matmul(z_ps[:], Th_sb[:], As[:], start=True, stop=True)
                nc.scalar.copy(ZU[:, 0:128], z_ps[:])
                big_ps = pc.tile([128, 132], f32, tag="pc132")
                nc.tensor.matmul(big_ps[:], As[:], ZU[:, 0:132], start=True, stop=True)
                nc.scalar.copy(YU[:], big_ps[:])
                for c in range(2):
                    t_ps = pc.tile([128, 132], f32, tag="pc132")
                    nc.tensor.matmul(
                        t_ps[:], WwT[:, c * 128 : (c + 1) * 128], YU[:, 0:132],
                        start=True, stop=True,
                    )
                    nc.scalar.copy(sc8[:, c * 4 : (c + 1) * 4], t_ps[:, 128:132])
                    scr = wp.tile([128, 128], f32, tag="qscr")
                    nc.vector.scalar_tensor_tensor(
                        out=scr[:], in0=t_ps[:, 0:128], scalar=0.0, in1=Ww[c][:],
                        op0=ALU.bypass, op1=ALU.mult,
                        accum_out=qdiag[:, c : c + 1],
                    )

                s1b = sc8[:, :].rearrange("p (c j) -> p j c", j=4)[:, 0, :]
                s2b = sc8[:, :].rearrange("p (c j) -> p j c", j=4)[:, 2, :]
                nc.vector.scalar_tensor_tensor(
                    out=stats[:, 0:2], in0=s2b, scalar=float(L), in1=s1b,
                    op0=ALU.mult, op1=ALU.add,
                )
                t1 = cp.tile([128, 2], f32, tag="qt")
                nc.vector.tensor_add(t1[:], s1b, stats[:, 0:2])
                nc.vector.tensor_mul(t1[:], t1[:], s2b)
                nc.vector.tensor_add(stats[:, 2:4], qdiag[:, :], t1[:])

                # ---- collective: AllGather on [4,128] payload ---------------
                # transposed so the gathered readback is 32 contiguous rows
                statsT = cp.tile([4, 128], f32, tag="statsT")
                stT_ps = sp.tile([4, 128], f32, tag="t")
                nc.tensor.transpose(stT_ps[:], stats[:], ident[:])
                nc.scalar.copy(statsT[:], stT_ps[:])
                cc_in = dp.tile([4, 128], f32)
                cc_out = dp.tile([4 * N_CORES, 128], f32)
                nc.sync.dma_start(cc_in[:], statsT[:])
                nc.gpsimd.collective_compute(
                    "AllGather",
                    ALU.bypass,
                    replica_groups=[list(range(N_CORES))],
                    ins=[cc_in[:].opt()],
                    outs=[cc_out[:].opt()],
                )
                g32 = cp.tile([32, 128], f32, tag="g32")
                nc.sync.dma_start(g32[:], cc_out[:])

                # ---- compositions (overlap CC) ------------------------------
                my_ps = pc.tile([128, C], f32, tag="pc256")
                nc.tensor.matmul(my_ps[:], As[:], thw_b[:], start=True, stop=True)
                nc.vector.tensor_copy(M_y[:], my_ps[:])
                for ci in range(2):
                    mw_ps = pc.tile([128, C], f32, tag="pc256")
                    nc.tensor.matmul(
                        mw_ps[:], M_y[:, ci * 128 : (ci + 1) * 128], WwT[:],
                        start=True, stop=True,
                    )
                    nc.vector.tensor_copy(M_wT[ci][:], mw_ps[:])

            # ---- phase 2: Wy = M_w x (overlaps CC) -------------------------
            with tc.tile_pool(name="pw", bufs=3, space="PSUM") as pw:
                ev = 0
                for k in range(NCH):
                    sl = slice(k * CH, (k + 1) * CH)
                    for co in range(2):
                        wps = pw.tile([128, CH], f32, tag="wy")
                        nc.tensor.matmul(
                            wps[:], M_wT[0][:, co * 128 : (co + 1) * 128],
                            xb[0][:, sl], start=True, stop=False,
                        )
                        nc.tensor.matmul(
                            wps[:], M_wT[1][:, co * 128 : (co + 1) * 128],
                            xb[1][:, sl], start=False, stop=True,
                        )
                        if ev % 2 == 0:
                            nc.scalar.copy(Wy[co][:, sl], wps[:])
                        else:
                            nc.vector.tensor_copy(Wy[co][:, sl], wps[:])
                        ev += 1

            # ---- BN coefficients -------------------------------------------
            me4 = cp.tile([128, 4], f32, tag="me4")
            with tc.tile_pool(name="pr", bufs=2, space="PSUM") as pr:
                tot4_ps = pr.tile([4, 128], f32, tag="tot4")
                nc.tensor.matmul(tot4_ps[:], sel32[:], g32[:], start=True, stop=True)
                tot4 = cp.tile([4, 128], f32, tag="tot4sb")
                nc.scalar.copy(tot4[:], tot4_ps[:])
                totT_ps = pr.tile([128, 4], f32, tag="totT")
                nc.tensor.transpose(totT_ps[:], tot4[:], ident[0:4, 0:4])
                nc.vector.tensor_scalar_mul(me4[:], totT_ps[:], 1.0 / (N_CORES * L))
            mean = me4[:, 0:2]
            ex2 = me4[:, 2:4]
            var = cp.tile([128, 2], f32, tag="var")
            nc.vector.tensor_mul(var[:], mean, mean)
            nc.vector.tensor_sub(var[:], ex2, var[:])
            sd = cp.tile([128, 2], f32, tag="sd")
            nc.scalar.activation(sd[:], var[:], AF.Sqrt, bias=epsc[:, 0:1])
            inv = cp.tile([128, 2], f32, tag="inv")
            nc.vector.reciprocal(inv[:], sd[:])
            a_sc = cp.tile([128, 2], f32, tag="a_sc")
            nc.vector.tensor_mul(a_sc[:], gam2[:], inv[:])
            s2b2 = sc8[:, :].rearrange("p (c j) -> p j c", j=4)[:, 2, :]
            b2 = cp.tile([128, 2], f32, tag="b2")
            nc.vector.tensor_sub(b2[:], s2b2, mean)
            nc.vector.tensor_mul(b2[:], b2[:], a_sc[:])
            nc.vector.tensor_add(b2[:], bet2[:], b2[:])

            # ---- tail: out = diag(a) Wy + I x (PE) + bias on evict ---------
            # PE applies the per-channel scale and residual in PSUM; the only
            # elementwise work left is the biased eviction (split
            # scalar/vector). Out DMA split in 2 pieces/unit so no engine
            # gets a serial straggler.
            diag_a = [cp.tile([128, 128], bf16, tag=f"dga{c}", name=f"dga{c}") for c in range(2)]
            for c in range(2):
                nc.vector.tensor_scalar_mul(diag_a[c][:], ident_b[:], a_sc[:, c : c + 1])
            with tc.tile_pool(name="po", bufs=5, space="PSUM") as po:
                un = 0
                for c in range(2):
                    for kp, gn in ((0, 3), (3, 3), (6, 2)):
                        sls = [slice((kp + i) * CH, (kp + i + 1) * CH) for i in range(gn)]
                        opss = [
                            po.tile([128, CH], f32, tag="ops", name=f"ops{i}")
                            for i in range(gn)
                        ]
                        for i in range(gn):
                            nc.tensor.matmul(
                                opss[i][:], diag_a[c][:], Wy[c][:, sls[i]],
                                start=True, stop=False,
                            )
                        for i in range(gn):
                            nc.tensor.matmul(
                                opss[i][:], ident_b[:], xb[c][:, sls[i]],
                                start=False, stop=True,
                            )
                        for i in range(gn):
                            to = wp.tile([128, CH], f32, tag="out", bufs=6)
                            if un % 2 == 0:
                                nc.scalar.activation(
                                    to[:], opss[i][:], AF.Identity,
                                    bias=b2[:, c : c + 1],
                                )
                            else:
                                nc.vector.tensor_scalar_add(
                                    to[:], opss[i][:], b2[:, c : c + 1]
                                )
                            for h in range(2):
                                p0 = c * 128 + h * 64
                                nc.sync.dma_start(
                                    out_e[p0 : p0 + 64, sls[i]],
                                    to[h * 64 : (h + 1) * 64, :],
                                )
                            un += 1

    nc.compile()
    return nc


_NC_CACHE = {}


def _get_nc():
    if "nc" not in _NC_CACHE:
        _NC_CACHE["nc"] = build_nc()
    return _NC_CACHE["nc"]


def make_in_maps(x, theta_w, theta_b, phi_w, phi_b, g_w, g_b, W_w, gamma, beta):
    base = {
        "theta_w": np.ascontiguousarray(np.asarray(theta_w, dtype=np.float32)),
        "theta_b": np.ascontiguousarray(np.asarray(theta_b, dtype=np.float32).reshape(OC, 1)),
        "phi_w": np.ascontiguousarray(np.asarray(phi_w, dtype=np.float32)),
        "phi_b": np.ascontiguousarray(np.asarray(phi_b, dtype=np.float32).reshape(1, OC)),
        "g_w": np.ascontiguousarray(np.asarray(g_w, dtype=np.float32)),
        "g_b": np.ascontiguousarray(np.asarray(g_b, dtype=np.float32).reshape(1, OC)),
        "W_w": np.ascontiguousarray(np.asarray(W_w, dtype=np.float32)),
        "gamma": np.ascontiguousarray(np.asarray(gamma, dtype=np.float32).reshape(C, 1)),
        "beta": np.ascontiguousarray(np.asarray(beta, dtype=np.float32).reshape(C, 1)),
    }
    x = np.asarray(x, dtype=np.float32)
    return [dict(base, x=np.ascontiguousarray(x[i])) for i in range(N_CORES)]


def kernel(x, g_w, g_b, theta_w, theta_b, phi_w, phi_b, W_w, W_b, gamma, beta):
    nc = _get_nc()
    in_maps = make_in_maps(x, theta_w, theta_b, phi_w, phi_b, g_w, g_b, W_w, gamma, beta)
    res = run_bass_kernel_spmd(nc, in_maps, core_ids=list(range(N_CORES)))
    return np.stack([res.results[i]["out"] for i in range(N_CORES)])
